# revision 1
# baseline (speedup 1.0000x reference)
"""Trainium2 Bass kernel for the LSTM discriminator.

Model: LSTM(H=720) over x[B=2048, T=256, F=51], keep last hidden state,
then sigmoid -> Dense(1024) -> LeakyReLU(0.3) -> Dense(256) -> LeakyReLU(0.3)
-> Dense(1).

Strategy:
  * Data parallel over 8 NeuronCores: 256 batch rows per core; all weights
    replicated.
  * Everything on-device is computed in the *transposed* layout: states and
    activations are [feature, batch] so the per-step recurrence matmul
    z^T = Wh^T h^T + Wx^T x_t^T needs no transposes in the loop (weights in
    natural layout serve directly as lhsT).
  * Per core the 256 batch rows are split into two independent chains of 128.
    The chains are interleaved step by step so the TensorE matmuls of one
    chain overlap the ScalarE (sigmoid/tanh) + VectorE (cell update) work of
    the other chain.
  * Matmuls run in bf16 (1 cycle/row on the PE vs 4 for fp32) with fp32 PSUM
    accumulation; the cell state c stays fp32.
  * The input projection Wx^T x_t is fused into the same PSUM accumulation
    group as the recurrence. x_t^T is zero-padded to K=120 so every matmul
    in the loop has the same contraction size (uniform K avoids a PE
    pipeline hiccup at accumulation-group starts); the LSTM bias rides
    along as a ones-row of x_t^T at partition 96.
  * x arrives in natural [batch, t, feature] layout and is transposed on the
    PE (one 128x51 transpose per chain-step, ~107 ns) via the identity
    matmul trick.
  * The head (sigmoid + 3 dense layers) runs in fp32: bf16 there dominated
    the end-to-end error (the outputs have small magnitude), and the head is
    only ~66 matmuls (~20 us).
  * A ~10 us burst of dummy matmuls right after the weight DMAs warms the
    PE's HAM clock gate to 2.4 GHz before the recurrence starts (otherwise
    the whole loop can run at the cold 1.2 GHz).
"""

import os
import sys

import numpy as np

_TRN = "/opt/trn_rl_repo"
if _TRN not in sys.path:
    sys.path.insert(0, _TRN)

import ml_dtypes  # noqa: E402

import concourse.bacc as bacc  # noqa: E402
import concourse.tile as tile  # noqa: E402
from concourse import mybir  # noqa: E402
from concourse.bass_utils import run_bass_kernel_spmd  # noqa: E402

F32 = mybir.dt.float32
F32R = mybir.dt.float32r
# head matmuls in plain fp32 (4 cycles/row on the PE, but the head is only
# ~66 matmuls so the cost is ~15us; fp32 keeps the head's error ~0 where
# bf16 there dominated the end-to-end error). float32r crashes walrus.
HEAD_DT = F32
BF16 = mybir.dt.bfloat16
AF = mybir.ActivationFunctionType
ALU = mybir.AluOpType

B, T_FULL, F, H = 2048, 256, 51, 720
D1, D2 = 1024, 256
NCORES = 8
BSH = B // NCORES  # 256 batch rows per core
NB = 128           # batch rows per chain (2 chains per core)
HJ, NJ = 120, 6    # H = 720 split into 6 chunks of 120 (partition dim)
G4 = 4 * H         # 2880
KX = 120           # x rows zero-padded to uniform K (=HJ); bias ones-row at ONES_ROW
ONES_ROW = 96
TC = 32            # timesteps of x staged per DMA chunk

_NC_CACHE = {}
LAST_EXEC_NS = None
LAST_RESULTS = None


def _build(T):
    nc = bacc.Bacc(
        "TRN2", target_bir_lowering=False, debug=False, enable_asserts=False
    )

    xa_d = nc.dram_tensor("xa", [NB, T * F], F32, kind="ExternalInput").ap()
    xb_d = nc.dram_tensor("xb", [NB, T * F], F32, kind="ExternalInput").ap()
    wh_d = nc.dram_tensor("wh", [NJ, HJ, G4], BF16, kind="ExternalInput").ap()
    wxb_d = nc.dram_tensor("wxb", [KX, G4], BF16, kind="ExternalInput").ap()
    w1_d = nc.dram_tensor("w1", [NJ, HJ, D1], F32, kind="ExternalInput").ap()
    w2_d = nc.dram_tensor("w2", [8, 128, D2], F32, kind="ExternalInput").ap()
    w3_d = nc.dram_tensor("w3", [2, 128, 1], F32, kind="ExternalInput").ap()
    b1_d = nc.dram_tensor("b1t", [128, 8], F32, kind="ExternalInput").ap()
    b2_d = nc.dram_tensor("b2t", [128, 2], F32, kind="ExternalInput").ap()
    b3_d = nc.dram_tensor("b3t", [1, 1], F32, kind="ExternalInput").ap()
    id_d = nc.dram_tensor("ident", [128, 128], F32, kind="ExternalInput").ap()
    out_d = nc.dram_tensor("out", [BSH, 1], F32, kind="ExternalOutput").ap()

    import contextlib

    with tile.TileContext(nc) as tc, contextlib.ExitStack() as ctx:
        if T > 16:
            tc.race_detector_enabled = False

        const = ctx.enter_context(tc.tile_pool(name="const", bufs=1))
        xpool = ctx.enter_context(tc.tile_pool(name="xp", bufs=2))
        gpool = ctx.enter_context(tc.tile_pool(name="gp", bufs=3))
        spool = ctx.enter_context(tc.tile_pool(name="st", bufs=1))
        zpool = ctx.enter_context(tc.tile_pool(name="zp", bufs=6, space="PSUM"))
        tpool = ctx.enter_context(tc.tile_pool(name="tp", bufs=2, space="PSUM"))

        # ---- weights / constants into SBUF ----
        # DMA order matters: identity + x chunks first (the warmup matmuls
        # and first transposes depend on them), then the LSTM weights, then
        # head weights (needed only at the very end).
        id_t = const.tile([128, 128], F32, tag="id", name="id")
        nc.sync.dma_start(id_t[:], id_d[:])
        wxb_t = const.tile([KX, G4], BF16, tag="wxb", name="wxb")
        wh_t = const.tile([HJ, NJ * G4], BF16, tag="wh", name="wh")
        w1_t = const.tile([HJ, NJ * D1], F32, tag="w1", name="w1")
        w2_t = const.tile([128, 8 * D2], F32, tag="w2", name="w2")
        w3_t = const.tile([128, 2], F32, tag="w3", name="w3")
        b1_t = const.tile([128, 8], F32, tag="b1", name="b1")
        b2_t = const.tile([128, 2], F32, tag="b2", name="b2")
        b3_t = const.tile([1, 1], F32, tag="b3", name="b3")

        def load_weights():
            # spread the big weight loads across several engines' DMA queues
            # so they run in parallel instead of serializing behind x
            qs = [nc.gpsimd, nc.scalar, nc.sync]
            qi = [0]

            def dma(dst, src):
                qs[qi[0] % len(qs)].dma_start(dst, src)
                qi[0] += 1

            dma(wxb_t[:], wxb_d[:])
            for j in range(NJ):
                dma(wh_t[:, j * G4 : (j + 1) * G4], wh_d[j])
            for j in range(NJ):
                dma(w1_t[:, j * D1 : (j + 1) * D1], w1_d[j])
            for k in range(8):
                dma(w2_t[:, k * D2 : (k + 1) * D2], w2_d[k])
            for k in range(2):
                dma(w3_t[:, k : k + 1], w3_d[k])
            dma(b1_t[:], b1_d[:])
            dma(b2_t[:], b2_d[:])
            dma(b3_t[:], b3_d[:])

        # ---- persistent state ----
        # h, c, in transposed layout: [HJ, NJ * NB]; column block j holds
        # feature rows [120j, 120j+120) for the chain's 128 batch cols.
        hT = [
            [spool.tile([HJ, NJ * NB], BF16, tag=f"h{c}{p}", name=f"h{c}{p}") for p in range(2)]
            for c in range(2)
        ]
        cT = [spool.tile([HJ, NJ * NB], F32, tag=f"c{c}", name=f"c{c}") for c in range(2)]
        xT = [spool.tile([KX, NB], BF16, tag=f"xT{c}", name=f"xT{c}") for c in range(2)]
        for c in range(2):
            nc.vector.memset(hT[c][0][:], 0.0)
            nc.vector.memset(cT[c][:], 0.0)
            # rows 0..F-1 are overwritten by the per-step transpose copy;
            # rows F..KX-1 stay 0 except the bias ones-row at ONES_ROW.
            # Zero-padding x to K=120 keeps every matmul in the loop at the
            # same contraction size (no PE pipeline disruption at group
            # starts from K changes).
            nc.vector.memset(xT[c][:], 0.0)
            nc.vector.memset(xT[c][ONES_ROW : ONES_ROW + 1, :], 1.0)

        xd = [xa_d, xb_d]
        nchunks = (T + TC - 1) // TC
        xtiles = [[None] * nchunks for _ in range(2)]

        def ensure_chunk(c, ch):
            if ch >= nchunks or xtiles[c][ch] is not None:
                return
            sz = min(TC, T - ch * TC)
            t_ = xpool.tile([NB, TC * F], F32, tag=f"xc{c}", name=f"xc{c}")
            nc.sync.dma_start(
                t_[:, : sz * F], xd[c][:, ch * TC * F : (ch * TC + sz) * F]
            )
            xtiles[c][ch] = t_

        ensure_chunk(0, 0)
        ensure_chunk(1, 0)
        load_weights()

        def emit_transpose(s):
            c, t = s % 2, s // 2
            ch, off = t // TC, t % TC
            ensure_chunk(c, ch)
            if off == 0:
                ensure_chunk(c, ch + 1)  # prefetch the next chunk early
            tp = tpool.tile([F, NB], F32, tag="tp", name="tp")
            nc.tensor.transpose(
                tp[:], xtiles[c][ch][:, off * F : (off + 1) * F], id_t[:]
            )
            nc.vector.tensor_copy(xT[c][0:F, :], tp[:])

        def emit_chain_step(s):
            c, t = s % 2, s // 2
            p = t % 2
            h_rd = hT[c][p]
            h_wr = hT[c][1 - p]
            for jlist in ((0, 1, 2, 3), (4, 5)):
                W = NB * len(jlist)
                c0 = NB * jlist[0]
                zt = []
                for g in range(4):
                    z = zpool.tile([HJ, W], F32, tag="z", name="z")
                    for ii, j in enumerate(jlist):
                        mc = 720 * g + HJ * j
                        o0 = ii * NB
                        nc.tensor.matmul(
                            z[:, o0 : o0 + NB],
                            wxb_t[:, mc : mc + HJ],
                            xT[c][:],
                            start=True,
                            stop=False,
                        )
                        for k in range(NJ):
                            nc.tensor.matmul(
                                z[:, o0 : o0 + NB],
                                wh_t[:, k * G4 + mc : k * G4 + mc + HJ],
                                h_rd[:, k * NB : (k + 1) * NB],
                                start=False,
                                stop=(k == NJ - 1),
                            )
                    zt.append(z)
                si = gpool.tile([HJ, W], F32, tag="si", name="si")
                nc.scalar.activation(si[:], zt[0][:], AF.Sigmoid)
                sf = gpool.tile([HJ, W], F32, tag="sf", name="sf")
                nc.scalar.activation(sf[:], zt[1][:], AF.Sigmoid)
                tg = gpool.tile([HJ, W], F32, tag="tg", name="tg")
                nc.scalar.activation(tg[:], zt[2][:], AF.Tanh)
                so = gpool.tile([HJ, W], F32, tag="so", name="so")
                nc.scalar.activation(so[:], zt[3][:], AF.Sigmoid)
                csl = cT[c][:, c0 : c0 + W]
                t1 = gpool.tile([HJ, W], F32, tag="t1", name="t1")
                nc.vector.tensor_mul(t1[:], sf[:], csl)
                t2 = gpool.tile([HJ, W], F32, tag="t2", name="t2")
                nc.vector.tensor_mul(t2[:], si[:], tg[:])
                nc.vector.tensor_add(csl, t1[:], t2[:])
                tq = gpool.tile([HJ, W], F32, tag="tc", name="tc")
                nc.scalar.activation(tq[:], csl, AF.Tanh)
                nc.vector.tensor_mul(h_wr[:, c0 : c0 + W], so[:], tq[:])

        # HAM warmup: ~6us of dense matmul work reading wh (the last big DMA),
        # so it runs right before the loop with no idle gap and flips the PE
        # clock gate to 8/8 (2.4 GHz) before the recurrence starts.
        # (reads the LAST wh block so it starts only after all LSTM weights
        # are resident and runs flush against the loop start)
        wm = zpool.tile([128, 512], F32, tag="z", name="wm")
        w0 = (NJ - 1) * G4
        for w_ in range(16):
            nc.tensor.matmul(
                wm[:],
                wh_t[:, w0 : w0 + 128],
                wh_t[:, w0 + 1024 : w0 + 1536],
                start=True,
                stop=True,
            )

        emit_transpose(0)
        S = 2 * T
        for s in range(S):
            if s + 1 < S:
                emit_transpose(s + 1)
            emit_chain_step(s)

        # ---- head: sigmoid -> FC1+leaky -> FC2+leaky -> FC3 ----
        pfin = T % 2
        sgh = spool.tile([HJ, NJ * BSH], F32, tag="sgh", name="sgh")
        for j in range(NJ):
            for c in range(2):
                d0 = j * BSH + c * NB
                nc.scalar.activation(
                    sgh[:, d0 : d0 + NB],
                    hT[c][pfin][:, j * NB : (j + 1) * NB],
                    AF.Sigmoid,
                )
        o1 = spool.tile([128, 8 * BSH], F32, tag="o1", name="o1")
        for m in range(8):
            ps = zpool.tile([128, BSH], F32, tag="z", name="z")
            for j in range(NJ):
                nc.tensor.matmul(
                    ps[:],
                    w1_t[:, j * D1 + m * 128 : j * D1 + (m + 1) * 128].bitcast(HEAD_DT),
                    sgh[:, j * BSH : (j + 1) * BSH].bitcast(HEAD_DT),
                    start=(j == 0),
                    stop=(j == NJ - 1),
                )
            tb = gpool.tile([128, BSH], F32, tag="hb", name="hb")
            nc.vector.tensor_scalar_add(tb[:], ps[:], b1_t[:, m : m + 1])
            nc.vector.scalar_tensor_tensor(
                o1[:, m * BSH : (m + 1) * BSH], tb[:], 0.3, tb[:], ALU.mult, ALU.max
            )
        o2 = spool.tile([128, 2 * BSH], F32, tag="o2", name="o2")
        for m in range(2):
            ps = zpool.tile([128, BSH], F32, tag="z", name="z")
            for k in range(8):
                nc.tensor.matmul(
                    ps[:],
                    w2_t[:, k * D2 + m * 128 : k * D2 + (m + 1) * 128].bitcast(HEAD_DT),
                    o1[:, k * BSH : (k + 1) * BSH].bitcast(HEAD_DT),
                    start=(k == 0),
                    stop=(k == 7),
                )
            tb = gpool.tile([128, BSH], F32, tag="hb", name="hb")
            nc.vector.tensor_scalar_add(tb[:], ps[:], b2_t[:, m : m + 1])
            nc.vector.scalar_tensor_tensor(
                o2[:, m * BSH : (m + 1) * BSH], tb[:], 0.3, tb[:], ALU.mult, ALU.max
            )
        ps = zpool.tile([1, BSH], F32, tag="z", name="z")
        for k in range(2):
            nc.tensor.matmul(
                ps[:],
                w3_t[:, k : k + 1].bitcast(HEAD_DT),
                o2[:, k * BSH : (k + 1) * BSH].bitcast(HEAD_DT),
                start=(k == 0),
                stop=(k == 1),
            )
        ob = spool.tile([1, BSH], F32, tag="ob", name="ob")
        nc.vector.tensor_scalar_add(ob[:], ps[:], b3_t[:])
        nc.sync.dma_start(out_d[:], ob[:])

    nc.compile()
    return nc


def _get_nc(T):
    if T not in _NC_CACHE:
        _NC_CACHE[T] = _build(T)
    return _NC_CACHE[T]


def kernel(x, Wx, Wh, b, W1, b1, W2, b2, W3, b3):
    global LAST_EXEC_NS, LAST_RESULTS
    x = np.asarray(x, dtype=np.float32)
    T = x.shape[1]
    nc = _get_nc(T)

    bf = ml_dtypes.bfloat16
    wh = np.ascontiguousarray(np.asarray(Wh, np.float32).reshape(NJ, HJ, G4)).astype(bf)
    wxb = np.zeros((KX, G4), np.float32)
    wxb[:F] = np.asarray(Wx, np.float32)
    wxb[ONES_ROW] = np.asarray(b, np.float32)
    wxb = wxb.astype(bf)
    w1 = np.ascontiguousarray(np.asarray(W1, np.float32).reshape(NJ, HJ, D1))
    w2 = np.ascontiguousarray(np.asarray(W2, np.float32).reshape(8, 128, D2))
    w3 = np.ascontiguousarray(np.asarray(W3, np.float32).reshape(2, 128, 1))
    b1t = np.ascontiguousarray(np.asarray(b1, np.float32).reshape(8, 128).T)
    b2t = np.ascontiguousarray(np.asarray(b2, np.float32).reshape(2, 128).T)
    b3t = np.asarray(b3, np.float32).reshape(1, 1)
    ident = np.eye(128, dtype=np.float32)

    shared = {
        "wh": wh,
        "wxb": wxb,
        "w1": w1,
        "w2": w2,
        "w3": w3,
        "b1t": b1t,
        "b2t": b2t,
        "b3t": b3t,
        "ident": ident,
    }
    in_maps = []
    for i in range(NCORES):
        xs = x[i * BSH : (i + 1) * BSH]
        in_maps.append(
            {
                "xa": np.ascontiguousarray(xs[:NB].reshape(NB, T * F)),
                "xb": np.ascontiguousarray(xs[NB:].reshape(NB, T * F)),
                **shared,
            }
        )

    trace = bool(os.environ.get("KLSTM_TRACE"))
    res = run_bass_kernel_spmd(nc, in_maps, list(range(NCORES)), trace=trace)
    LAST_RESULTS = res
    LAST_EXEC_NS = res.exec_time_ns
    out = np.concatenate([r["out"] for r in res.results], axis=0)
    return out.astype(np.float32)

